# revision 11
# baseline (speedup 1.0000x reference)
"""Trainium2 Bass kernel for nn_Decoder_23991687315866.

Two stacked LSTM cells applied independently per (t, b) (the reference
re-feeds the same initial state at every horizon step), preceded by three
tiny embedding lookups concatenated with dec_x.

Strategy (pure data parallel over B=4096 -> 512 rows/core on 8 cores):
  host:  - fold the 3 embedding tables through W_ih0 into one combined
           1000x20 gate-space table; gather it per (t,b) and ship it
           transposed, fused with dec_x^T and the t-invariant bias rows:
           xcat[T, 72, B_loc] = [dec_x^T(32); base0^T(20); e^T(20)]
         - precompute base_l = h_l @ W_hh_l.T + b_ih + b_hh
  device: per t-step, 4 batch-chunks of 128 rows:
         - L0 gates in ONE self-loading bf16 matmul per chunk:
           lhsT = xcat column block (72 x 128), rhs = [Wd.T; I20; I20]
           -> row-major PSUM gates [128 rows, 20]
         - gate nonlinearities on ACT, products on DVE/GPSIMD, batched
           over 8 t-steps (4096 rows per op)
         - h1 written into a [128, G*128] bf16 tile, one HWDGE
           DMA-transpose per batch -> [128, G, 128]; per t one K=128
           matmul with block-diagonal rhs computes all 4 chunks' L1 gates
         - + resident base1, gate math again -> h2 -> row-major DMA out
"""

import sys

for _p in ("/opt/trn_rl_repo", "/root/.axon_site/_ro/trn_rl_repo"):
    if _p not in sys.path:
        sys.path.append(_p)

import numpy as np
from contextlib import ExitStack

import ml_dtypes

T, BL, C, H = 64, 512, 4, 5  # time, batch/core, 128-row chunks, hidden
G = 8                        # t-steps per elementwise batch
NB = T // G
N_CORES = 8
BF16 = ml_dtypes.bfloat16

_CACHE = {}


def build_nc(reps=None):
    import concourse.bacc as bacc
    import concourse.tile as tile
    import concourse.bass as bass
    from concourse import mybir

    f32 = mybir.dt.float32
    bf16 = mybir.dt.bfloat16
    Sig = mybir.ActivationFunctionType.Sigmoid
    Tanh = mybir.ActivationFunctionType.Tanh
    mult = mybir.AluOpType.mult
    add = mybir.AluOpType.add

    nc = bacc.Bacc("TRN2", target_bir_lowering=False, debug=False,
                   enable_asserts=True, num_devices=N_CORES)

    xcat = nc.dram_tensor("xcat", [T, 72, BL], bf16, kind="ExternalInput").ap()
    base1 = nc.dram_tensor("base1", [128, C, 20], bf16, kind="ExternalInput").ap()
    cell0 = nc.dram_tensor("cell0", [128, C, H], f32, kind="ExternalInput").ap()
    cell1 = nc.dram_tensor("cell1", [128, C, H], f32, kind="ExternalInput").ap()
    w0 = nc.dram_tensor("w0", [72, 20], bf16, kind="ExternalInput").ap()
    w1 = nc.dram_tensor("w1", [128, C * 20], bf16, kind="ExternalInput").ap()
    out = nc.dram_tensor("out", [T, BL, H], f32, kind="ExternalOutput").ap()

    def bcast_g(ap, n, after=1):
        # insert a stride-0 dim of size n after `after` leading dims
        a = ap.ap
        return bass.AP(tensor=ap.tensor, offset=ap.offset,
                       ap=list(a[:after]) + [[0, n]] + list(a[after:]))

    with ExitStack() as ctx:
        tc = ctx.enter_context(tile.TileContext(nc))
        singles = ctx.enter_context(tc.tile_pool(name="singles", bufs=1))
        xp = ctx.enter_context(tc.tile_pool(name="xp", bufs=2))
        gp = ctx.enter_context(tc.tile_pool(name="gp", bufs=2))
        sp = ctx.enter_context(tc.tile_pool(name="sp", bufs=2))
        sm = ctx.enter_context(tc.tile_pool(name="sm", bufs=2))
        hp = ctx.enter_context(tc.tile_pool(name="hp", bufs=2))
        tp = ctx.enter_context(tc.tile_pool(name="tp", bufs=2))
        op_ = ctx.enter_context(tc.tile_pool(name="op", bufs=2))
        pp = ctx.enter_context(tc.tile_pool(name="pp", bufs=2, space="PSUM"))

        w0_sb = singles.tile([72, 20], bf16)
        nc.sync.dma_start(out=w0_sb[:], in_=w0[:])
        w1_sb = singles.tile([128, C * 20], bf16)
        nc.sync.dma_start(out=w1_sb[:], in_=w1[:])
        c0_sb = singles.tile([128, C, H], f32)
        nc.sync.dma_start(out=c0_sb[:], in_=cell0[:])
        c1_sb = singles.tile([128, C, H], f32)
        nc.sync.dma_start(out=c1_sb[:], in_=cell1[:])
        b1_sb = singles.tile([128, C, 20], bf16)
        nc.sync.dma_start(out=b1_sb[:], in_=base1[:])

        c0_b = bcast_g(c0_sb[:], G)   # [128, G, C, H] stride-0 over G
        c1_b = bcast_g(c1_sb[:], G)
        b1_b = bcast_g(b1_sb[:], G)   # [128, G, C, 20]

        if reps is not None:
            loop_ctx = ctx.enter_context(tc.For_i(
                0, reps, 1,
                hint_engines=(mybir.EngineType.PE, mybir.EngineType.SP,
                              mybir.EngineType.Activation,
                              mybir.EngineType.DVE)))

        for b in range(NB):
            t0 = b * G
            psum0 = pp.tile([128, G, C, 32], f32)
            psum1 = pp.tile([128, G, 128], f32)

            x_sb = xp.tile([72, G, BL], bf16)
            nc.sync.dma_start(
                out=x_sb[:],
                in_=xcat[t0:t0 + G].rearrange("g f p -> f g p"))

            h1aug = hp.tile([128, G, C, 32], bf16)
            nc.gpsimd.memset(h1aug[:, :, :, 5:32], 0.0)

            for it in range(G):
                for c in range(C):
                    nc.tensor.matmul(
                        out=psum0[:, it, c, 0:20],
                        lhsT=x_sb[:, it, c * 128:(c + 1) * 128],
                        rhs=w0_sb[:], start=True, stop=True)

            # ---- layer 0 gate math (gate order i,f,o,g) ----
            s0 = sp.tile([128, G, C, 15], bf16)
            nc.scalar.activation(out=s0[:], in_=psum0[:, :, :, 0:15], func=Sig)
            tg0 = sm.tile([128, G, C, H], bf16, tag="tg")
            nc.scalar.activation(out=tg0[:], in_=psum0[:, :, :, 15:20], func=Tanh)
            m0 = sm.tile([128, G, C, H], bf16, tag="m")
            nc.gpsimd.tensor_tensor(out=m0[:], in0=s0[:, :, :, 0:5], in1=tg0[:], op=mult)
            v0 = sm.tile([128, G, C, H], bf16, tag="v")
            nc.vector.tensor_tensor(out=v0[:], in0=s0[:, :, :, 5:10], in1=c0_b, op=mult)
            cc0 = sm.tile([128, G, C, H], bf16, tag="cc")
            nc.vector.tensor_tensor(out=cc0[:], in0=m0[:], in1=v0[:], op=add)
            tc0 = sm.tile([128, G, C, H], bf16, tag="tc")
            nc.scalar.activation(out=tc0[:], in_=cc0[:], func=Tanh)
            nc.vector.tensor_tensor(out=h1aug[:, :, :, 0:5],
                                    in0=s0[:, :, :, 10:15], in1=tc0[:], op=mult)

            # ---- layer 1: one transpose per batch, one matmul per t ----
            h1T = tp.tile([128, G, 128], bf16)
            nc.sync.dma_start_transpose(
                out=h1T[:], in_=h1aug[:].rearrange("p g c k -> p (g c k)"))
            for it in range(G):
                nc.tensor.matmul(
                    out=psum1[:, it, 0:C * 20], lhsT=h1T[:, it, :],
                    rhs=w1_sb[:], start=True, stop=True)

            # ---- layer 1 gate math ----
            psum1_v = psum1[:, :, 0:C * 20].rearrange("p g (c k) -> p g c k", k=20)
            g1 = gp.tile([128, G, C, 20], bf16)
            nc.vector.tensor_tensor(out=g1[:], in0=psum1_v, in1=b1_b, op=add)
            s1 = sp.tile([128, G, C, 15], bf16)
            nc.scalar.activation(out=s1[:], in_=g1[:, :, :, 0:15], func=Sig)
            tg1 = sm.tile([128, G, C, H], bf16, tag="tg")
            nc.scalar.activation(out=tg1[:], in_=g1[:, :, :, 15:20], func=Tanh)
            m1 = sm.tile([128, G, C, H], bf16, tag="m")
            nc.gpsimd.tensor_tensor(out=m1[:], in0=s1[:, :, :, 0:5], in1=tg1[:], op=mult)
            v1 = sm.tile([128, G, C, H], bf16, tag="v")
            nc.vector.tensor_tensor(out=v1[:], in0=s1[:, :, :, 5:10], in1=c1_b, op=mult)
            cc1 = sm.tile([128, G, C, H], bf16, tag="cc")
            nc.vector.tensor_tensor(out=cc1[:], in0=m1[:], in1=v1[:], op=add)
            tc1 = sm.tile([128, G, C, H], bf16, tag="tc")
            nc.scalar.activation(out=tc1[:], in_=cc1[:], func=Tanh)
            h2 = op_.tile([128, G, C, H], f32)
            nc.vector.tensor_tensor(out=h2[:], in0=s1[:, :, :, 10:15], in1=tc1[:], op=mult)

            for c in range(C):
                out_view = bass.AP(
                    tensor=out.tensor,
                    offset=out.offset + (t0 * BL + 128 * c) * H,
                    ap=[[H, 128], [BL * H, G], [1, H]])
                nc.sync.dma_start(out=out_view, in_=h2[:, :, c, :])

    nc.compile()
    return nc


def prep_inputs(horizon, hidden, cell, dec_x, mote_id_cat, fault_type_cat,
                mote_fault_cat, mote_embed, W_ih0, W_hh0, b_ih0, b_hh0,
                W_ih1, W_hh1, b_ih1, b_hh1):
    hidden = np.asarray(hidden, np.float32)
    cell = np.asarray(cell, np.float32)
    dec_x = np.asarray(dec_x, np.float32)
    mote_embed = np.asarray(mote_embed, np.float32)
    W_ih0 = np.asarray(W_ih0, np.float32)
    W_hh0 = np.asarray(W_hh0, np.float32)
    W_ih1 = np.asarray(W_ih1, np.float32)
    W_hh1 = np.asarray(W_hh1, np.float32)
    b0 = np.asarray(b_ih0, np.float32) + np.asarray(b_hh0, np.float32)
    b1 = np.asarray(b_ih1, np.float32) + np.asarray(b_hh1, np.float32)

    perm = np.r_[0:5, 5:10, 15:20, 10:15]  # [i,f,g,o] -> [i,f,o,g]

    Wd = W_ih0[perm][:, 0:32]                       # [20, 32]
    M1 = mote_embed @ W_ih0[perm][:, 32:64].T       # [10, 20]
    M2 = mote_embed @ W_ih0[perm][:, 64:96].T
    M3 = mote_embed @ W_ih0[perm][:, 96:128].T
    mc = (M3[:, None, None, :] + M2[None, :, None, :]
          + M1[None, None, :, :]).reshape(1000, 20)  # idx = a + 10b + 100c
    base0 = hidden[0] @ W_hh0[perm].T + b0[perm]     # [4096, 20]
    base1 = hidden[1] @ W_hh1[perm].T + b1[perm]

    idxc = (np.asarray(mote_id_cat, np.int64)
            + 10 * np.asarray(fault_type_cat, np.int64)
            + 100 * np.asarray(mote_fault_cat, np.int64)).astype(np.int32)  # [T, 4096]

    w0 = np.concatenate([Wd.T, np.eye(20, dtype=np.float32),
                         np.eye(20, dtype=np.float32)], axis=0)  # [72, 20]
    w1 = np.zeros((128, C, 20), np.float32)
    for c in range(C):
        w1[32 * c:32 * c + 5, c] = W_ih1[perm].T
    w1 = w1.reshape(128, C * 20)

    w0_b = w0.astype(BF16)
    w1_b = w1.astype(BF16)
    mc_f = mc  # bf16 rounding happens once when xcat is cast below

    in_maps = []
    for k in range(N_CORES):
        s = slice(k * BL, (k + 1) * BL)
        # xcat[t] = [dec_x[t].T (32); base0.T (20); e[t].T (20)]
        xc = np.empty((T, 72, BL), np.float32)
        xc[:, 0:32] = dec_x[:, s, :].transpose(0, 2, 1)
        xc[:, 32:52] = base0[s].T[None]
        xc[:, 52:72] = mc_f[idxc[:, s]].transpose(0, 2, 1)
        in_maps.append(dict(
            xcat=xc.astype(BF16),
            base1=np.ascontiguousarray(
                base1[s].reshape(C, 128, 20).transpose(1, 0, 2)).astype(BF16),
            cell0=np.ascontiguousarray(
                cell[0, s].reshape(C, 128, H).transpose(1, 0, 2)),
            cell1=np.ascontiguousarray(
                cell[1, s].reshape(C, 128, H).transpose(1, 0, 2)),
            w0=w0_b, w1=w1_b,
        ))
    return in_maps


def kernel(**inputs):
    from concourse import bass_utils
    if "nc" not in _CACHE:
        _CACHE["nc"] = build_nc()
    nc = _CACHE["nc"]
    in_maps = prep_inputs(**inputs)
    res = bass_utils.run_bass_kernel_spmd(nc, in_maps, core_ids=list(range(N_CORES)))
    full = np.concatenate([res.results[k]["out"] for k in range(N_CORES)], axis=1)
    T_h = int(inputs["horizon"])
    return np.ascontiguousarray(full[:T_h]).astype(np.float32)


# revision 14
# speedup vs baseline: 1.2403x; 1.2403x over previous
"""Trainium2 Bass kernel for nn_Decoder_23991687315866.

Two stacked LSTM cells applied independently per (t, b) (the reference
re-feeds the same initial state at every horizon step), preceded by three
tiny embedding lookups concatenated with dec_x.

Strategy (pure data parallel over B=4096 -> 512 rows/core on 8 cores):
  host:  - fold the 3 embedding tables through W_ih0 into one combined
           1000x20 gate-space table, gather it per (t,b), add the
           t-invariant base0 = h0 @ W_hh0.T + b -> be0[T, B, 20]
         - pack per-core device input xall[T, 128, 208] =
           [dec_x chunk-transposed (4 chunks x 32 feats) ; be0 (4 x 20)]
  device per t-step (512 rows = 4 chunks of 128 lanes):
         - L0: ONE K=128 block-diagonal bf16 matmul (lhsT = stacked
           chunk-transposed dec_x, rhs selects each chunk's 32 features)
           -> row-major PSUM gates [128, 4, 20]; one DVE add of be0
         - gate nonlinearities on ACT, products on DVE/GPSIMD, batched
           over 8 t-steps (4096 rows per op)
         - h1 -> [128, G*128] bf16 tile, one HWDGE DMA-transpose per
           batch -> [128, G, 128]; L1 = one K=128 block-diagonal matmul
           per t; + resident base1; gate math again -> h2 -> DMA out
"""

import sys

for _p in ("/opt/trn_rl_repo", "/root/.axon_site/_ro/trn_rl_repo"):
    if _p not in sys.path:
        sys.path.append(_p)

import numpy as np
from contextlib import ExitStack

import ml_dtypes

T, BL, C, H = 64, 512, 4, 5  # time, batch/core, 128-row chunks, hidden
G = 8                        # t-steps per elementwise batch
NB = T // G
N_CORES = 8
BF16 = ml_dtypes.bfloat16

_CACHE = {}


def build_nc(reps=None):
    import concourse.bacc as bacc
    import concourse.tile as tile
    import concourse.bass as bass
    from concourse import mybir

    f32 = mybir.dt.float32
    bf16 = mybir.dt.bfloat16
    Sig = mybir.ActivationFunctionType.Sigmoid
    Tanh = mybir.ActivationFunctionType.Tanh
    mult = mybir.AluOpType.mult
    add = mybir.AluOpType.add

    nc = bacc.Bacc("TRN2", target_bir_lowering=False, debug=False,
                   enable_asserts=True, num_devices=N_CORES)

    xall = nc.dram_tensor("xall", [T, 128, 208], bf16, kind="ExternalInput").ap()
    base1 = nc.dram_tensor("base1", [128, C, 20], bf16, kind="ExternalInput").ap()
    cell0 = nc.dram_tensor("cell0", [128, C, H], f32, kind="ExternalInput").ap()
    cell1 = nc.dram_tensor("cell1", [128, C, H], f32, kind="ExternalInput").ap()
    w0 = nc.dram_tensor("w0", [128, C * 20], bf16, kind="ExternalInput").ap()
    w1 = nc.dram_tensor("w1", [128, C * 20], bf16, kind="ExternalInput").ap()
    out = nc.dram_tensor("out", [T, BL, H], f32, kind="ExternalOutput").ap()

    def bcast_g(ap, n, after=1):
        # insert a stride-0 dim of size n after `after` leading dims
        a = ap.ap
        return bass.AP(tensor=ap.tensor, offset=ap.offset,
                       ap=list(a[:after]) + [[0, n]] + list(a[after:]))

    with ExitStack() as ctx:
        tc = ctx.enter_context(tile.TileContext(nc))
        singles = ctx.enter_context(tc.tile_pool(name="singles", bufs=1))
        xp = ctx.enter_context(tc.tile_pool(name="xp", bufs=3))
        gp = ctx.enter_context(tc.tile_pool(name="gp", bufs=2))
        g1p = ctx.enter_context(tc.tile_pool(name="g1p", bufs=2))
        sp = ctx.enter_context(tc.tile_pool(name="sp", bufs=2))
        sm = ctx.enter_context(tc.tile_pool(name="sm", bufs=2))
        hp = ctx.enter_context(tc.tile_pool(name="hp", bufs=2))
        tp = ctx.enter_context(tc.tile_pool(name="tp", bufs=2))
        op_ = ctx.enter_context(tc.tile_pool(name="op", bufs=2))
        pp = ctx.enter_context(tc.tile_pool(name="pp", bufs=2, space="PSUM"))

        w0_sb = singles.tile([128, C * 20], bf16)
        nc.sync.dma_start(out=w0_sb[:], in_=w0[:])
        w1_sb = singles.tile([128, C * 20], bf16)
        nc.sync.dma_start(out=w1_sb[:], in_=w1[:])
        c0_sb = singles.tile([128, C, H], f32)
        nc.sync.dma_start(out=c0_sb[:], in_=cell0[:])
        c1_sb = singles.tile([128, C, H], f32)
        nc.sync.dma_start(out=c1_sb[:], in_=cell1[:])
        b1_sb = singles.tile([128, C, 20], bf16)
        nc.sync.dma_start(out=b1_sb[:], in_=base1[:])

        h1aug = singles.tile([128, G, C, 32], bf16)
        nc.gpsimd.memset(h1aug[:, :, :, 5:32], 0.0)

        c0_b = bcast_g(c0_sb[:], G)   # [128, G, C, H] stride-0 over G
        c1_b = bcast_g(c1_sb[:], G)
        b1_b = bcast_g(b1_sb[:], G)   # [128, G, C, 20]

        if reps is not None:
            loop_ctx = ctx.enter_context(tc.For_i(
                0, reps, 1,
                hint_engines=(mybir.EngineType.PE, mybir.EngineType.SP,
                              mybir.EngineType.Activation,
                              mybir.EngineType.DVE, mybir.EngineType.Pool)))

        h2 = None
        for b in range(NB):
            t0 = b * G
            psum0 = pp.tile([128, G, 128], f32, tag="ps0")
            psum1 = pp.tile([128, G, 128], f32, tag="ps1")

            x_sb = xp.tile([128, G, 208], bf16)
            nc.sync.dma_start(
                out=x_sb[:], in_=xall[t0:t0 + G].rearrange("g p f -> p g f"))
            be0 = x_sb[:, :, 128:208].rearrange("p g (c k) -> p g c k", k=20)

            for it in range(G):
                nc.tensor.matmul(
                    out=psum0[:, it, 0:C * 20],
                    lhsT=x_sb[:, it, 0:128],
                    rhs=w0_sb[:], start=True, stop=True)

            # ---- layer 0 gate math (gate order i,f,o,g) ----
            psum0_v = psum0[:, :, 0:C * 20].rearrange("p g (c k) -> p g c k", k=20)
            g0 = gp.tile([128, G, C, 20], bf16)
            nc.vector.tensor_tensor(out=g0[:], in0=psum0_v, in1=be0, op=add)
            s0 = sp.tile([128, G, C, 15], bf16, tag="s")
            nc.scalar.activation(out=s0[:], in_=g0[:, :, :, 0:15], func=Sig)
            tg0 = sm.tile([128, G, C, H], bf16, tag="tg")
            nc.scalar.activation(out=tg0[:], in_=g0[:, :, :, 15:20], func=Tanh)
            m0 = sm.tile([128, G, C, H], bf16, tag="m")
            nc.gpsimd.tensor_tensor(out=m0[:], in0=s0[:, :, :, 0:5], in1=tg0[:], op=mult)
            v0 = sm.tile([128, G, C, H], bf16, tag="v")
            nc.gpsimd.tensor_tensor(out=v0[:], in0=s0[:, :, :, 5:10], in1=c0_b, op=mult)
            cc0 = sm.tile([128, G, C, H], bf16, tag="cc")
            nc.vector.tensor_tensor(out=cc0[:], in0=m0[:], in1=v0[:], op=add)
            tc0 = sm.tile([128, G, C, H], bf16, tag="tc")
            nc.scalar.activation(out=tc0[:], in_=cc0[:], func=Tanh)
            nc.vector.tensor_tensor(out=h1aug[:, :, :, 0:5],
                                    in0=s0[:, :, :, 10:15], in1=tc0[:], op=mult)

            # ---- layer 1: one transpose per batch, one matmul per t ----
            h1T = tp.tile([128, G, 128], bf16)
            nc.sync.dma_start_transpose(
                out=h1T[:], in_=h1aug[:].rearrange("p g c k -> p (g c k)"))
            for it in range(G):
                nc.tensor.matmul(
                    out=psum1[:, it, 0:C * 20], lhsT=h1T[:, it, :],
                    rhs=w1_sb[:], start=True, stop=True)

            # ---- layer 1 gate math ----
            psum1_v = psum1[:, :, 0:C * 20].rearrange("p g (c k) -> p g c k", k=20)
            g1 = g1p.tile([128, G, C, 20], bf16)
            nc.vector.tensor_tensor(out=g1[:], in0=psum1_v, in1=b1_b, op=add)
            s1 = sp.tile([128, G, C, 15], bf16, tag="s")
            nc.scalar.activation(out=s1[:], in_=g1[:, :, :, 0:15], func=Sig)
            tg1 = sm.tile([128, G, C, H], bf16, tag="tg")
            nc.scalar.activation(out=tg1[:], in_=g1[:, :, :, 15:20], func=Tanh)
            m1 = sm.tile([128, G, C, H], bf16, tag="m")
            nc.gpsimd.tensor_tensor(out=m1[:], in0=s1[:, :, :, 0:5], in1=tg1[:], op=mult)
            v1 = sm.tile([128, G, C, H], bf16, tag="v")
            nc.gpsimd.tensor_tensor(out=v1[:], in0=s1[:, :, :, 5:10], in1=c1_b, op=mult)
            cc1 = sm.tile([128, G, C, H], bf16, tag="cc")
            nc.vector.tensor_tensor(out=cc1[:], in0=m1[:], in1=v1[:], op=add)
            tc1 = sm.tile([128, G, C, H], bf16, tag="tc")
            nc.scalar.activation(out=tc1[:], in_=cc1[:], func=Tanh)
            if b % 2 == 0:
                h2 = op_.tile([128, 2, G, C, H], f32)
            nc.vector.tensor_tensor(out=h2[:, b % 2], in0=s1[:, :, :, 10:15],
                                    in1=tc1[:], op=mult)

            if b % 2 == 1:
                for c in range(C):
                    out_view = bass.AP(
                        tensor=out.tensor,
                        offset=out.offset + ((t0 - G) * BL + 128 * c) * H,
                        ap=[[H, 128], [BL * H, 2 * G], [1, H]])
                    nc.sync.dma_start(
                        out=out_view,
                        in_=h2[:].rearrange("p two g c k -> p (two g) c k")[:, :, c, :])

    nc.compile()
    return nc


def prep_inputs(horizon, hidden, cell, dec_x, mote_id_cat, fault_type_cat,
                mote_fault_cat, mote_embed, W_ih0, W_hh0, b_ih0, b_hh0,
                W_ih1, W_hh1, b_ih1, b_hh1):
    hidden = np.asarray(hidden, np.float32)
    cell = np.asarray(cell, np.float32)
    dec_x = np.asarray(dec_x, np.float32)
    mote_embed = np.asarray(mote_embed, np.float32)
    W_ih0 = np.asarray(W_ih0, np.float32)
    W_hh0 = np.asarray(W_hh0, np.float32)
    W_ih1 = np.asarray(W_ih1, np.float32)
    W_hh1 = np.asarray(W_hh1, np.float32)
    b0 = np.asarray(b_ih0, np.float32) + np.asarray(b_hh0, np.float32)
    b1 = np.asarray(b_ih1, np.float32) + np.asarray(b_hh1, np.float32)

    perm = np.r_[0:5, 5:10, 15:20, 10:15]  # [i,f,g,o] -> [i,f,o,g]

    Wd = W_ih0[perm][:, 0:32]                       # [20, 32]
    M1 = mote_embed @ W_ih0[perm][:, 32:64].T       # [10, 20]
    M2 = mote_embed @ W_ih0[perm][:, 64:96].T
    M3 = mote_embed @ W_ih0[perm][:, 96:128].T
    mc = (M3[:, None, None, :] + M2[None, :, None, :]
          + M1[None, None, :, :]).reshape(1000, 20)  # idx = a + 10b + 100c
    base0 = hidden[0] @ W_hh0[perm].T + b0[perm]     # [4096, 20]
    base1 = hidden[1] @ W_hh1[perm].T + b1[perm]

    idxc = (np.asarray(mote_id_cat, np.int64)
            + 10 * np.asarray(fault_type_cat, np.int64)
            + 100 * np.asarray(mote_fault_cat, np.int64)).astype(np.int32)  # [T, 4096]

    w0b = np.zeros((128, C, 20), np.float32)
    w1b = np.zeros((128, C, 20), np.float32)
    for c in range(C):
        w0b[32 * c:32 * c + 32, c] = Wd.T
        w1b[32 * c:32 * c + 5, c] = W_ih1[perm].T
    w0_b = w0b.reshape(128, C * 20).astype(BF16)
    w1_b = w1b.reshape(128, C * 20).astype(BF16)

    in_maps = []
    for k in range(N_CORES):
        s = slice(k * BL, (k + 1) * BL)
        # xall[t] rows 32c+f (f<32): dec_x[t, 128c+lane, f]
        xa = np.empty((T, 128, 208), np.float32)
        xa[:, :, 0:128] = dec_x[:, s, :].reshape(T, C, 128, 32).transpose(
            0, 1, 3, 2).reshape(T, 128, 128)
        be0 = mc[idxc[:, s]] + base0[s][None]        # [T, 512, 20]
        xa[:, :, 128:208] = be0.reshape(T, C, 128, 20).transpose(
            0, 2, 1, 3).reshape(T, 128, C * 20)
        in_maps.append(dict(
            xall=xa.astype(BF16),
            base1=np.ascontiguousarray(
                base1[s].reshape(C, 128, 20).transpose(1, 0, 2)).astype(BF16),
            cell0=np.ascontiguousarray(
                cell[0, s].reshape(C, 128, H).transpose(1, 0, 2)),
            cell1=np.ascontiguousarray(
                cell[1, s].reshape(C, 128, H).transpose(1, 0, 2)),
            w0=w0_b, w1=w1_b,
        ))
    return in_maps


def kernel(**inputs):
    from concourse import bass_utils
    if "nc" not in _CACHE:
        _CACHE["nc"] = build_nc()
    nc = _CACHE["nc"]
    in_maps = prep_inputs(**inputs)
    res = bass_utils.run_bass_kernel_spmd(nc, in_maps, core_ids=list(range(N_CORES)))
    full = np.concatenate([res.results[k]["out"] for k in range(N_CORES)], axis=1)
    T_h = int(inputs["horizon"])
    return np.ascontiguousarray(full[:T_h]).astype(np.float32)
